# revision 15
# baseline (speedup 1.0000x reference)
"""Fused RNN cell on 8 Trainium2 NeuronCores.

Reference computation (fp32):
    combined   = [x, hidden]                      [B=4096, I+H=4096]
    new_hidden = tanh(combined @ W_ih^T + b_ih)   [B, H=2048]
    output     = new_hidden @ W_ho^T + b_ho       [B, O=2048]
    returns (output, new_hidden)

Strategy: data-parallel over the batch — each of the 8 cores processes 512
batch rows with replicated weights; no collectives. All operand layout
transforms (transposes into PE-friendly [K-partition, free] form) happen on
the host so every device DMA is a fat, fully contiguous transfer:

    c   [128, 32, 512]      cL[ki, ko, b]       = combined[b, ko*128+ki]
    w1  [128, 32, 16, 128]  w1L[ki, ko, hc, h]  = W_ih[hc*128+h, ko*128+ki]
    w2  [128, 16, 16, 128]  w2L[hi, ho, oc, o]  = W_ho[oc*128+o, ho*128+hi]
    b1  [128, 16]           b1L[p, hc]          = b_ih[hc*128+p]

All matmul operands are fp16 (full PE rate, 1 col/cycle; fp32 PSUM
accumulation; rms rel err ~5e-4), which halves HBM traffic vs fp32 —
28 MB in + 4 MB out per core vs the ~166 us PE floor, so the kernel is
cleanly compute-bound.

DMA is split across the two HWDGE rings so neither saturates and the
first tiles land ASAP: the sync ring carries only weight loads (w1, w2);
the scalar (ACT) ring carries c loads, nh stores, and out stores. The
first four w1 slices are 1-ko (256 KB) so the PE can start ~1.5 us
earlier than with 2-ko slices; after the pipeline fills, 2-ko slices
(0.5 MB) keep trigger overhead low. Dummy matmuls at t=0 warm the PE
clock gate (HAM, ~3.4 us busy to reach 2.4 GHz) during the initial DMA
ramp, and a 1-element tanh preloads the ACT table set (~2.7 us) behind
the first c triggers.

mm1 drains: 8 back-to-back tanh ACTs per PSUM group (bias fused), then
the 8 nh store triggers — triggers never sit between ACT ops, so bank
turnaround stays at copy cadence and the next group never stalls. mm2
drains alternate DVE/ACT copies the same way; all out-store triggers
ride the scalar ring after the group's copies. mm2 groups are [8, 6, 2]
so only two small stores remain after the final matmul.
"""

import numpy as np

import concourse.bass as bass
import concourse.mybir as mybir
import concourse.tile as tile
from concourse import bacc, bass_utils

NCORES = 8
B, I, H, O = 4096, 2048, 2048, 2048
BC = B // NCORES          # 512 batch rows per core
K1 = I + H                # mm1 contraction dim, 4096
KO1 = K1 // 128           # 32 k-chunks for mm1
HC = H // 128             # 16 h-chunks
OC = O // 128             # 16 o-chunks
G = 8                     # h-chunks per PSUM group (8 banks)
P = 128
F32 = mybir.dt.float32
F16 = mybir.dt.float16
AF = mybir.ActivationFunctionType
NWARM = 33                # dummy matmuls covering the DMA ramp (~107ns each)


def _build():
    nc = bacc.Bacc("TRN2", target_bir_lowering=False)

    c = nc.dram_tensor("c", [P, KO1, BC], F16, kind="ExternalInput")
    w1 = nc.dram_tensor("w1", [P, KO1, HC, P], F16, kind="ExternalInput")
    b1 = nc.dram_tensor("b1", [P, HC], F32, kind="ExternalInput")
    w2 = nc.dram_tensor("w2", [P, HC, OC, P], F16, kind="ExternalInput")
    nhT = nc.dram_tensor("nhT", [H, BC], F16, kind="ExternalOutput")
    outT = nc.dram_tensor("outT", [O, BC], F16, kind="ExternalOutput")

    with tile.TileContext(nc) as tc:
        with tc.tile_pool(name="cpool", bufs=1) as cpool, \
             tc.tile_pool(name="wpool", bufs=10) as wpool, \
             tc.tile_pool(name="nhpool", bufs=1) as nhpool, \
             tc.tile_pool(name="opool", bufs=8) as opool, \
             tc.tile_pool(name="bpool", bufs=1) as bpool, \
             tc.tile_pool(name="ps", bufs=8, space="PSUM") as ps:

            b1_sb = bpool.tile([P, HC], F32)
            # b_ih isn't needed until the first group drains; keep it off
            # both HWDGE rings entirely (SWDGE via GpSimd). b_ho is added
            # on the host after the gather.
            nc.gpsimd.dma_start(b1_sb[:], b1[:])

            c_sb = cpool.tile([P, KO1, BC], F16)
            nh_sb = nhpool.tile([P, HC, BC], F16)

            # Warm-up operand, memset on GpSimd (whose queue opens first)
            # so the PE warm-ups start the instant the PE queue opens.
            warm_sb = bpool.tile([P, P], mybir.dt.bfloat16)
            nc.gpsimd.memset(warm_sb[:], 0.0)

            # Head: the first c chunk rides the sync ring (lowest
            # first-byte latency) ahead of the first weight slices; the
            # rest of c goes on the scalar ring, whose first chunks beat
            # the ACT tanh-table preload (~2.7 us) into the queue.
            nc.sync.dma_start(c_sb[:, 0:1], c[:, 0:1])
            nc.scalar.dma_start(c_sb[:, 1:2], c[:, 1:2])
            nc.scalar.dma_start(c_sb[:, 2:4], c[:, 2:4])
            nc.scalar.dma_start(c_sb[:, 4:6], c[:, 4:6])
            act_warm = bpool.tile([1, 1], F32)
            nc.scalar.activation(act_warm[:], warm_sb[:1, :1], AF.Tanh)
            for ko0 in range(6, KO1, 2):
                nc.scalar.dma_start(c_sb[:, ko0:ko0 + 2], c[:, ko0:ko0 + 2])

            # mm1: nh^T[h, b] = tanh(W_ih @ combined^T + b_ih)
            # Two G-sized PSUM groups ping-pong across the 8 banks.
            for g in range(HC // G):
                psums = [ps.tile([P, BC], F32, tag="ps", name=f"ps{i}")
                         for i in range(G)]
                if g == 0:
                    # PE warm-up: HAM holds the PE at 1.2 GHz until ~3.4 us
                    # of busy time. Dummy matmuls (into the last bank this
                    # group will touch; start=True on the real group clears
                    # it) keep the PE active while the first tiles stream
                    # in, so real matmuls run near 2.4 GHz from the start.
                    for _ in range(NWARM):
                        nc.tensor.matmul(
                            psums[G - 1][:, :P], lhsT=warm_sb[:],
                            rhs=warm_sb[:],
                            start=True, stop=True, skip_group_check=True,
                        )
                    # Ramp-sized slices: two 128 KB half-width pieces for
                    # ko=0, then 256 KB 1-ko slices, then steady-state
                    # 0.5 MB 2-ko slices once the ring pipeline is full.
                    slices = [(0, 1, 0, 4), (0, 1, 4, 4),
                              (1, 1, 0, G), (2, 1, 0, G), (3, 1, 0, G)] + \
                             [(ko, 2, 0, G) for ko in range(4, KO1, 2)]
                else:
                    slices = [(ko, 2, 0, G) for ko in range(0, KO1, 2)]
                for ko0, kw, i0, ni in slices:
                    w1_sb = wpool.tile([P, 2, G, P], F16, tag="w")
                    nc.sync.dma_start(
                        w1_sb[:, :kw, :ni],
                        w1[:, ko0:ko0 + kw, g * G + i0:g * G + i0 + ni])
                    for kk in range(kw):
                        for i in range(ni):
                            nc.tensor.matmul(
                                psums[i0 + i][:],
                                lhsT=w1_sb[:, kk, i],
                                rhs=c_sb[:, ko0 + kk],
                                start=(ko0 + kk == 0),
                                stop=(ko0 + kk == KO1 - 1),
                            )
                # Back-to-back tanhs first (bank turnaround at ACT copy
                # cadence), store triggers after.
                for i in range(G):
                    hc = g * G + i
                    nc.scalar.activation(
                        nh_sb[:, hc], psums[i][:], AF.Tanh,
                        bias=b1_sb[:, hc:hc + 1],
                    )
                for i in range(G):
                    hc = g * G + i
                    nc.scalar.dma_start(
                        nhT[hc * P:(hc + 1) * P, :], nh_sb[:, hc])

            # mm2: out^T[o, b] = W_ho @ nh^T (+ b_ho on host)
            # Groups of [8, 6, 2] o-chunks: consecutive groups ping-pong
            # through the 8 PSUM banks, and the final drain after the last
            # matmul is just two chunks on two parallel rings.
            for g0, gsz in ((0, 8), (8, 6), (14, 2)):
                psums = [ps.tile([P, BC], F32, tag="ps", name=f"ps{i}")
                         for i in range(gsz)]
                for ho0 in range(0, HC, 2):
                    w2_sb = wpool.tile([P, 2, G, P], F16, tag="w",
                                       name="w2_sb")[:, :, :gsz]
                    nc.sync.dma_start(
                        w2_sb[:], w2[:, ho0:ho0 + 2, g0:g0 + gsz])
                    for kk in range(2):
                        for i in range(gsz):
                            nc.tensor.matmul(
                                psums[i][:],
                                lhsT=w2_sb[:, kk, i],
                                rhs=nh_sb[:, ho0 + kk],
                                start=(ho0 + kk == 0),
                                stop=(ho0 + kk == HC - 1),
                            )
                # Alternate DVE/ACT copies back-to-back, then the store
                # triggers (scalar ring; the sync ring takes one of the two
                # tail stores so the final drain runs on parallel rings).
                o_sbs = []
                for i in range(gsz):
                    o_sb = opool.tile([P, BC], F16, tag="osb")
                    if i % 2:
                        nc.scalar.activation(o_sb[:], psums[i][:], AF.Copy)
                    else:
                        nc.vector.tensor_copy(o_sb[:], psums[i][:])
                    o_sbs.append(o_sb)
                for i in range(gsz):
                    oc = g0 + i
                    # Final two stores ride parallel rings: the last chunk
                    # takes the faster sync ring (w2 loads are done).
                    eng = nc.sync if (gsz == 2 and i == 1) else nc.scalar
                    eng.dma_start(outT[oc * P:(oc + 1) * P, :], o_sbs[i][:])

    nc.compile()
    return nc


def _shard_inputs(x, hidden, W_ih, b_ih, W_ho, b_ho):
    combined = np.concatenate([x, hidden], axis=1)  # [B, K1]
    w1L = np.ascontiguousarray(
        W_ih.reshape(HC, P, KO1, P).transpose(3, 2, 0, 1).astype(np.float16)
    )  # [ki, ko, hc, h]
    w2L = np.ascontiguousarray(
        W_ho.reshape(OC, P, HC, P).transpose(3, 2, 0, 1).astype(np.float16)
    )  # [hi, ho, oc, o]
    b1L = np.ascontiguousarray(b_ih.reshape(HC, P).T)
    in_maps = []
    for cix in range(NCORES):
        cc = combined[cix * BC:(cix + 1) * BC]  # [BC, K1]
        cL = np.ascontiguousarray(
            cc.reshape(BC, KO1, P).transpose(2, 1, 0).astype(np.float16))
        in_maps.append(
            {"c": cL, "w1": w1L, "b1": b1L, "w2": w2L}
        )
    return in_maps


def _run(in_maps, **kwargs):
    nc = _build()
    return bass_utils.run_bass_kernel_spmd(
        nc, in_maps, core_ids=list(range(NCORES)), **kwargs
    )


def kernel(x, hidden, W_ih, b_ih, W_ho, b_ho):
    x = np.asarray(x, dtype=np.float32)
    hidden = np.asarray(hidden, dtype=np.float32)
    W_ih = np.asarray(W_ih, dtype=np.float32)
    b_ih = np.asarray(b_ih, dtype=np.float32)
    W_ho = np.asarray(W_ho, dtype=np.float32)
    b_ho = np.asarray(b_ho, dtype=np.float32)

    in_maps = _shard_inputs(x, hidden, W_ih, b_ih, W_ho, b_ho)
    res = _run(in_maps)
    output = np.concatenate(
        [r["outT"].T.astype(np.float32) for r in res.results], axis=0) + b_ho
    new_hidden = np.concatenate(
        [r["nhT"].T.astype(np.float32) for r in res.results], axis=0)
    return output, new_hidden


# revision 20
# speedup vs baseline: 1.0052x; 1.0052x over previous
"""Fused RNN cell on 8 Trainium2 NeuronCores.

Reference computation (fp32):
    combined   = [x, hidden]                      [B=4096, I+H=4096]
    new_hidden = tanh(combined @ W_ih^T + b_ih)   [B, H=2048]
    output     = new_hidden @ W_ho^T + b_ho       [B, O=2048]
    returns (output, new_hidden)

Strategy: data-parallel over the batch — each of the 8 cores processes 512
batch rows with replicated weights; no collectives. All operand layout
transforms (transposes into PE-friendly [K-partition, free] form) happen on
the host so every device DMA is a fat, fully contiguous transfer:

    c   [128, 32, 512]      cL[ki, ko, b]       = combined[b, ko*128+ki]
    w1  [128, 32, 16, 128]  w1L[ki, ko, hc, h]  = W_ih[hc*128+h, ko*128+ki]
    w2  [128, 16, 16, 128]  w2L[hi, ho, oc, o]  = W_ho[oc*128+o, ho*128+hi]
    b1  [128, 16]           b1L[p, hc]          = b_ih[hc*128+p]

All matmul operands are fp16 (full PE rate, 1 col/cycle; fp32 PSUM
accumulation; rms rel err ~5e-4), which halves HBM traffic vs fp32 —
28 MB in + 4 MB out per core vs the ~166 us PE floor, so the kernel is
cleanly compute-bound.

DMA is split across the two HWDGE rings so neither saturates and the
first tiles land ASAP: the sync ring carries only weight loads (w1, w2);
the scalar (ACT) ring carries c loads, nh stores, and out stores. The
first four w1 slices are 1-ko (256 KB) so the PE can start ~1.5 us
earlier than with 2-ko slices; after the pipeline fills, 2-ko slices
(0.5 MB) keep trigger overhead low. Dummy matmuls at t=0 warm the PE
clock gate (HAM, ~3.4 us busy to reach 2.4 GHz) during the initial DMA
ramp, and a 1-element tanh preloads the ACT table set (~2.7 us) behind
the first c triggers.

mm1 drains: 8 back-to-back tanh ACTs per PSUM group (bias fused), then
the 8 nh store triggers — triggers never sit between ACT ops, so bank
turnaround stays at copy cadence and the next group never stalls. mm2
drains alternate DVE/ACT copies the same way; all out-store triggers
ride the scalar ring after the group's copies. mm2 groups are [8, 6, 2]
so only two small stores remain after the final matmul.
"""

import numpy as np

import concourse.bass as bass
import concourse.mybir as mybir
import concourse.tile as tile
from concourse import bacc, bass_utils

NCORES = 8
B, I, H, O = 4096, 2048, 2048, 2048
BC = B // NCORES          # 512 batch rows per core
K1 = I + H                # mm1 contraction dim, 4096
KO1 = K1 // 128           # 32 k-chunks for mm1
HC = H // 128             # 16 h-chunks
OC = O // 128             # 16 o-chunks
G = 8                     # h-chunks per PSUM group (8 banks)
P = 128
F32 = mybir.dt.float32
F16 = mybir.dt.float16
AF = mybir.ActivationFunctionType
NWARM = 36                # dummy matmuls covering the DMA ramp (~107ns each)


def _build():
    nc = bacc.Bacc("TRN2", target_bir_lowering=False)

    c = nc.dram_tensor("c", [P, KO1, BC], F16, kind="ExternalInput")
    w1 = nc.dram_tensor("w1", [P, KO1, HC, P], F16, kind="ExternalInput")
    b1 = nc.dram_tensor("b1", [P, HC], F32, kind="ExternalInput")
    w2 = nc.dram_tensor("w2", [P, HC, OC, P], F16, kind="ExternalInput")
    nhT = nc.dram_tensor("nhT", [H, BC], F16, kind="ExternalOutput")
    outT = nc.dram_tensor("outT", [O, BC], F16, kind="ExternalOutput")

    with tile.TileContext(nc) as tc:
        with tc.tile_pool(name="cpool", bufs=1) as cpool, \
             tc.tile_pool(name="wpool", bufs=10) as wpool, \
             tc.tile_pool(name="nhpool", bufs=1) as nhpool, \
             tc.tile_pool(name="opool", bufs=8) as opool, \
             tc.tile_pool(name="bpool", bufs=1) as bpool, \
             tc.tile_pool(name="ps", bufs=8, space="PSUM") as ps:

            # Warm-up operand first: the sooner the memset lands, the
            # sooner the PE warm-ups can start spinning up the clock.
            warm_sb = bpool.tile([P, P], mybir.dt.bfloat16)
            nc.vector.memset(warm_sb[:], 0.0)

            b1_sb = bpool.tile([P, HC], F32)
            # b_ih isn't needed until the first group drains; keep it off
            # both HWDGE rings entirely (SWDGE via GpSimd). b_ho is added
            # on the host after the gather.
            nc.gpsimd.dma_start(b1_sb[:], b1[:])

            c_sb = cpool.tile([P, KO1, BC], F16)
            nh_sb = nhpool.tile([P, HC, BC], F16)

            # Scalar-ring head: the first c chunks (needed by the first
            # real matmuls; the very first as its own small transfer so
            # matmuls can start before the rest lands) go out before the
            # ACT tanh-table preload blocks the queue for ~2.7 us.
            nc.scalar.dma_start(c_sb[:, 0:1], c[:, 0:1])
            nc.scalar.dma_start(c_sb[:, 1:2], c[:, 1:2])
            nc.scalar.dma_start(c_sb[:, 2:4], c[:, 2:4])
            nc.scalar.dma_start(c_sb[:, 4:6], c[:, 4:6])
            act_warm = bpool.tile([1, 1], F32)
            nc.scalar.activation(act_warm[:], warm_sb[:1, :1], AF.Tanh)
            for ko0 in range(6, KO1, 2):
                nc.scalar.dma_start(c_sb[:, ko0:ko0 + 2], c[:, ko0:ko0 + 2])

            # mm1: nh^T[h, b] = tanh(W_ih @ combined^T + b_ih)
            # Two G-sized PSUM groups ping-pong across the 8 banks.
            for g in range(HC // G):
                psums = [ps.tile([P, BC], F32, tag="ps", name=f"ps{i}")
                         for i in range(G)]
                if g == 0:
                    # PE warm-up: HAM holds the PE at 1.2 GHz until ~3.4 us
                    # of busy time. Dummy matmuls (into the last bank this
                    # group will touch; start=True on the real group clears
                    # it) keep the PE active while the first tiles stream
                    # in, so real matmuls run near 2.4 GHz from the start.
                    for _ in range(NWARM):
                        nc.tensor.matmul(
                            psums[G - 1][:, :P], lhsT=warm_sb[:],
                            rhs=warm_sb[:],
                            start=True, stop=True, skip_group_check=True,
                        )
                    # Ramp-sized slices: 256 KB 1-ko slices while the ring
                    # pipeline fills, then steady-state 0.5 MB 2-ko ones.
                    slices = [(ko, 1) for ko in range(4)] + \
                             [(ko, 2) for ko in range(4, KO1, 2)]
                else:
                    slices = [(ko, 2) for ko in range(0, KO1, 2)]
                for ko0, kw in slices:
                    w1_sb = wpool.tile([P, 2, G, P], F16, tag="w")
                    nc.sync.dma_start(
                        w1_sb[:, :kw], w1[:, ko0:ko0 + kw, g * G:(g + 1) * G])
                    for kk in range(kw):
                        for i in range(G):
                            nc.tensor.matmul(
                                psums[i][:],
                                lhsT=w1_sb[:, kk, i],
                                rhs=c_sb[:, ko0 + kk],
                                start=(ko0 + kk == 0),
                                stop=(ko0 + kk == KO1 - 1),
                            )
                # Back-to-back tanhs first (bank turnaround at ACT copy
                # cadence), store triggers after.
                for i in range(G):
                    hc = g * G + i
                    nc.scalar.activation(
                        nh_sb[:, hc], psums[i][:], AF.Tanh,
                        bias=b1_sb[:, hc:hc + 1],
                    )
                for i in range(G):
                    hc = g * G + i
                    nc.scalar.dma_start(
                        nhT[hc * P:(hc + 1) * P, :], nh_sb[:, hc])

            # mm2: out^T[o, b] = W_ho @ nh^T (+ b_ho on host)
            # Groups of [8, 6, 2] o-chunks: consecutive groups ping-pong
            # through the 8 PSUM banks, and the final drain after the last
            # matmul is just two chunks on two parallel rings.
            for g0, gsz in ((0, 8), (8, 6), (14, 2)):
                psums = [ps.tile([P, BC], F32, tag="ps", name=f"ps{i}")
                         for i in range(gsz)]
                for ho0 in range(0, HC, 2):
                    w2_sb = wpool.tile([P, 2, G, P], F16, tag="w",
                                       name="w2_sb")[:, :, :gsz]
                    nc.sync.dma_start(
                        w2_sb[:], w2[:, ho0:ho0 + 2, g0:g0 + gsz])
                    for kk in range(2):
                        for i in range(gsz):
                            nc.tensor.matmul(
                                psums[i][:],
                                lhsT=w2_sb[:, kk, i],
                                rhs=nh_sb[:, ho0 + kk],
                                start=(ho0 + kk == 0),
                                stop=(ho0 + kk == HC - 1),
                            )
                # Alternate DVE/ACT copies back-to-back, then the store
                # triggers (scalar ring; the sync ring takes one of the two
                # tail stores so the final drain runs on parallel rings).
                o_sbs = []
                for i in range(gsz):
                    o_sb = opool.tile([P, BC], F16, tag="osb")
                    if i % 2:
                        nc.scalar.activation(o_sb[:], psums[i][:], AF.Copy)
                    else:
                        nc.vector.tensor_copy(o_sb[:], psums[i][:])
                    o_sbs.append(o_sb)
                for i in range(gsz):
                    oc = g0 + i
                    # Final two stores ride parallel rings: the last chunk
                    # takes the faster sync ring (w2 loads are done).
                    eng = nc.sync if (gsz == 2 and i == 1) else nc.scalar
                    eng.dma_start(outT[oc * P:(oc + 1) * P, :], o_sbs[i][:])

    nc.compile()
    return nc


def _shard_inputs(x, hidden, W_ih, b_ih, W_ho, b_ho):
    combined = np.concatenate([x, hidden], axis=1)  # [B, K1]
    w1L = np.ascontiguousarray(
        W_ih.reshape(HC, P, KO1, P).transpose(3, 2, 0, 1).astype(np.float16)
    )  # [ki, ko, hc, h]
    w2L = np.ascontiguousarray(
        W_ho.reshape(OC, P, HC, P).transpose(3, 2, 0, 1).astype(np.float16)
    )  # [hi, ho, oc, o]
    b1L = np.ascontiguousarray(b_ih.reshape(HC, P).T)
    in_maps = []
    for cix in range(NCORES):
        cc = combined[cix * BC:(cix + 1) * BC]  # [BC, K1]
        cL = np.ascontiguousarray(
            cc.reshape(BC, KO1, P).transpose(2, 1, 0).astype(np.float16))
        in_maps.append(
            {"c": cL, "w1": w1L, "b1": b1L, "w2": w2L}
        )
    return in_maps


def _run(in_maps, **kwargs):
    nc = _build()
    return bass_utils.run_bass_kernel_spmd(
        nc, in_maps, core_ids=list(range(NCORES)), **kwargs
    )


def kernel(x, hidden, W_ih, b_ih, W_ho, b_ho):
    x = np.asarray(x, dtype=np.float32)
    hidden = np.asarray(hidden, dtype=np.float32)
    W_ih = np.asarray(W_ih, dtype=np.float32)
    b_ih = np.asarray(b_ih, dtype=np.float32)
    W_ho = np.asarray(W_ho, dtype=np.float32)
    b_ho = np.asarray(b_ho, dtype=np.float32)

    in_maps = _shard_inputs(x, hidden, W_ih, b_ih, W_ho, b_ho)
    res = _run(in_maps)
    output = np.concatenate(
        [r["outT"].T.astype(np.float32) for r in res.results], axis=0) + b_ho
    new_hidden = np.concatenate(
        [r["nhT"].T.astype(np.float32) for r in res.results], axis=0)
    return output, new_hidden


# revision 23
# speedup vs baseline: 1.0076x; 1.0024x over previous
"""Fused RNN cell on 8 Trainium2 NeuronCores.

Reference computation (fp32):
    combined   = [x, hidden]                      [B=4096, I+H=4096]
    new_hidden = tanh(combined @ W_ih^T + b_ih)   [B, H=2048]
    output     = new_hidden @ W_ho^T + b_ho       [B, O=2048]
    returns (output, new_hidden)

Strategy: data-parallel over the batch — each of the 8 cores processes 512
batch rows with replicated weights; no collectives. All operand layout
transforms (transposes into PE-friendly [K-partition, free] form) happen on
the host so every device DMA is a fat, fully contiguous transfer:

    c   [128, 32, 512]      cL[ki, ko, b]       = combined[b, ko*128+ki]
    w1  [128, 32, 16, 128]  w1L[ki, ko, hc, h]  = W_ih[hc*128+h, ko*128+ki]
    w2  [128, 16, 16, 128]  w2L[hi, ho, oc, o]  = W_ho[oc*128+o, ho*128+hi]
    b1  [128, 16]           b1L[p, hc]          = b_ih[hc*128+p]

All matmul operands are fp16 (full PE rate, 1 col/cycle; fp32 PSUM
accumulation; rms rel err ~5e-4), which halves HBM traffic vs fp32 —
28 MB in + 4 MB out per core vs the ~166 us PE floor, so the kernel is
cleanly compute-bound.

DMA is split across the two HWDGE rings so neither saturates and the
first tiles land ASAP: the sync ring carries only weight loads (w1, w2);
the scalar (ACT) ring carries c loads, nh stores, and out stores. The
first four w1 slices are 1-ko (256 KB) so the PE can start ~1.5 us
earlier than with 2-ko slices; after the pipeline fills, 2-ko slices
(0.5 MB) keep trigger overhead low. Dummy matmuls at t=0 warm the PE
clock gate (HAM, ~3.4 us busy to reach 2.4 GHz) during the initial DMA
ramp, and a 1-element tanh preloads the ACT table set (~2.7 us) behind
the first c triggers.

mm1 drains: 8 back-to-back tanh ACTs per PSUM group (bias fused), then
the 8 nh store triggers — triggers never sit between ACT ops, so bank
turnaround stays at copy cadence and the next group never stalls. mm2
drains alternate DVE/ACT copies the same way; all out-store triggers
ride the scalar ring after the group's copies. mm2 groups are [8, 6, 2]
so only two small stores remain after the final matmul.
"""

import numpy as np

import concourse.bass as bass
import concourse.mybir as mybir
import concourse.tile as tile
from concourse import bacc, bass_utils

NCORES = 8
B, I, H, O = 4096, 2048, 2048, 2048
BC = B // NCORES          # 512 batch rows per core
K1 = I + H                # mm1 contraction dim, 4096
KO1 = K1 // 128           # 32 k-chunks for mm1
HC = H // 128             # 16 h-chunks
OC = O // 128             # 16 o-chunks
G = 8                     # h-chunks per PSUM group (8 banks)
P = 128
F32 = mybir.dt.float32
F16 = mybir.dt.float16
AF = mybir.ActivationFunctionType
NWARM = 32                # dummy matmuls covering the DMA ramp (~107ns each)


def _build():
    nc = bacc.Bacc("TRN2", target_bir_lowering=False)

    c = nc.dram_tensor("c", [P, KO1, BC], F16, kind="ExternalInput")
    w1 = nc.dram_tensor("w1", [P, KO1, HC, P], F16, kind="ExternalInput")
    b1 = nc.dram_tensor("b1", [P, HC], F32, kind="ExternalInput")
    w2 = nc.dram_tensor("w2", [P, HC, OC, P], F16, kind="ExternalInput")
    nhT = nc.dram_tensor("nhT", [H, BC], F16, kind="ExternalOutput")
    outT = nc.dram_tensor("outT", [O, BC], F16, kind="ExternalOutput")

    with tile.TileContext(nc) as tc:
        with tc.tile_pool(name="cpool", bufs=1) as cpool, \
             tc.tile_pool(name="wpool", bufs=10) as wpool, \
             tc.tile_pool(name="nhpool", bufs=1) as nhpool, \
             tc.tile_pool(name="opool", bufs=8) as opool, \
             tc.tile_pool(name="bpool", bufs=1) as bpool, \
             tc.tile_pool(name="ps", bufs=8, space="PSUM") as ps:

            # Warm-up operand first: the sooner the memset lands, the
            # sooner the PE warm-ups can start spinning up the clock.
            warm_sb = bpool.tile([P, P], mybir.dt.bfloat16)
            nc.vector.memset(warm_sb[:], 0.0)

            b1_sb = bpool.tile([P, HC], F32)

            c_sb = cpool.tile([P, KO1, BC], F16)
            nh_sb = nhpool.tile([P, HC, BC], F16)

            # Scalar-ring head: the first c chunks (needed by the first
            # real matmuls; the very first as its own small transfer so
            # matmuls can start before the rest lands) go out before the
            # ACT tanh-table preload blocks the queue for ~2.7 us.
            nc.scalar.dma_start(c_sb[:, 0:1], c[:, 0:1])
            nc.scalar.dma_start(c_sb[:, 1:2], c[:, 1:2])
            nc.scalar.dma_start(c_sb[:, 2:4], c[:, 2:4])
            nc.scalar.dma_start(c_sb[:, 4:6], c[:, 4:6])
            act_warm = bpool.tile([1, 1], F32)
            nc.scalar.activation(act_warm[:], warm_sb[:1, :1], AF.Tanh)
            for ko0 in range(6, KO1, 2):
                nc.scalar.dma_start(c_sb[:, ko0:ko0 + 2], c[:, ko0:ko0 + 2])
            # b_ih rides the scalar ring behind the c stream (it isn't
            # needed until the first group drains, ~60 us in); keeping it
            # off GpSimd SWDGE avoids the expensive dge_drain at teardown.
            nc.scalar.dma_start(b1_sb[:], b1[:])

            # mm1: nh^T[h, b] = tanh(W_ih @ combined^T + b_ih)
            # Two G-sized PSUM groups ping-pong across the 8 banks.
            for g in range(HC // G):
                psums = [ps.tile([P, BC], F32, tag="ps", name=f"ps{i}")
                         for i in range(G)]
                if g == 0:
                    # PE warm-up: HAM holds the PE at 1.2 GHz until ~3.4 us
                    # of busy time. Dummy matmuls (into the last bank this
                    # group will touch; start=True on the real group clears
                    # it) keep the PE active while the first tiles stream
                    # in, so real matmuls run near 2.4 GHz from the start.
                    for _ in range(NWARM):
                        nc.tensor.matmul(
                            psums[G - 1][:, :P], lhsT=warm_sb[:],
                            rhs=warm_sb[:],
                            start=True, stop=True, skip_group_check=True,
                        )
                    # Ramp-sized slices: 256 KB 1-ko slices while the ring
                    # pipeline fills, then steady-state 0.5 MB 2-ko ones.
                    slices = [(ko, 1) for ko in range(4)] + \
                             [(ko, 2) for ko in range(4, KO1, 2)]
                else:
                    slices = [(ko, 2) for ko in range(0, KO1, 2)]
                for ko0, kw in slices:
                    w1_sb = wpool.tile([P, 2, G, P], F16, tag="w")
                    nc.sync.dma_start(
                        w1_sb[:, :kw], w1[:, ko0:ko0 + kw, g * G:(g + 1) * G])
                    for kk in range(kw):
                        for i in range(G):
                            nc.tensor.matmul(
                                psums[i][:],
                                lhsT=w1_sb[:, kk, i],
                                rhs=c_sb[:, ko0 + kk],
                                start=(ko0 + kk == 0),
                                stop=(ko0 + kk == KO1 - 1),
                            )
                # Back-to-back tanhs first (bank turnaround at ACT copy
                # cadence), store triggers after.
                for i in range(G):
                    hc = g * G + i
                    nc.scalar.activation(
                        nh_sb[:, hc], psums[i][:], AF.Tanh,
                        bias=b1_sb[:, hc:hc + 1],
                    )
                for i in range(G):
                    hc = g * G + i
                    nc.scalar.dma_start(
                        nhT[hc * P:(hc + 1) * P, :], nh_sb[:, hc])

            # mm2: out^T[o, b] = W_ho @ nh^T (+ b_ho on host)
            # Groups of [8, 6, 2] o-chunks: consecutive groups ping-pong
            # through the 8 PSUM banks, and the final drain after the last
            # matmul is just two chunks on two parallel rings.
            for g0, gsz in ((0, 8), (8, 6), (14, 2)):
                psums = [ps.tile([P, BC], F32, tag="ps", name=f"ps{i}")
                         for i in range(gsz)]
                for ho0 in range(0, HC, 2):
                    w2_sb = wpool.tile([P, 2, G, P], F16, tag="w",
                                       name="w2_sb")[:, :, :gsz]
                    nc.sync.dma_start(
                        w2_sb[:], w2[:, ho0:ho0 + 2, g0:g0 + gsz])
                    for kk in range(2):
                        for i in range(gsz):
                            nc.tensor.matmul(
                                psums[i][:],
                                lhsT=w2_sb[:, kk, i],
                                rhs=nh_sb[:, ho0 + kk],
                                start=(ho0 + kk == 0),
                                stop=(ho0 + kk == HC - 1),
                            )
                # Alternate DVE/ACT copies back-to-back, then the store
                # triggers (scalar ring; the sync ring takes one of the two
                # tail stores so the final drain runs on parallel rings).
                o_sbs = []
                for i in range(gsz):
                    o_sb = opool.tile([P, BC], F16, tag="osb")
                    if i % 2:
                        nc.scalar.activation(o_sb[:], psums[i][:], AF.Copy)
                    else:
                        nc.vector.tensor_copy(o_sb[:], psums[i][:])
                    o_sbs.append(o_sb)
                for i in range(gsz):
                    oc = g0 + i
                    # Final two stores ride parallel rings: the last chunk
                    # takes the faster sync ring (w2 loads are done).
                    eng = nc.sync if (gsz == 2 and i == 1) else nc.scalar
                    eng.dma_start(outT[oc * P:(oc + 1) * P, :], o_sbs[i][:])

    nc.compile()
    return nc


def _shard_inputs(x, hidden, W_ih, b_ih, W_ho, b_ho):
    combined = np.concatenate([x, hidden], axis=1)  # [B, K1]
    w1L = np.ascontiguousarray(
        W_ih.reshape(HC, P, KO1, P).transpose(3, 2, 0, 1).astype(np.float16)
    )  # [ki, ko, hc, h]
    w2L = np.ascontiguousarray(
        W_ho.reshape(OC, P, HC, P).transpose(3, 2, 0, 1).astype(np.float16)
    )  # [hi, ho, oc, o]
    b1L = np.ascontiguousarray(b_ih.reshape(HC, P).T)
    in_maps = []
    for cix in range(NCORES):
        cc = combined[cix * BC:(cix + 1) * BC]  # [BC, K1]
        cL = np.ascontiguousarray(
            cc.reshape(BC, KO1, P).transpose(2, 1, 0).astype(np.float16))
        in_maps.append(
            {"c": cL, "w1": w1L, "b1": b1L, "w2": w2L}
        )
    return in_maps


def _run(in_maps, **kwargs):
    nc = _build()
    return bass_utils.run_bass_kernel_spmd(
        nc, in_maps, core_ids=list(range(NCORES)), **kwargs
    )


def kernel(x, hidden, W_ih, b_ih, W_ho, b_ho):
    x = np.asarray(x, dtype=np.float32)
    hidden = np.asarray(hidden, dtype=np.float32)
    W_ih = np.asarray(W_ih, dtype=np.float32)
    b_ih = np.asarray(b_ih, dtype=np.float32)
    W_ho = np.asarray(W_ho, dtype=np.float32)
    b_ho = np.asarray(b_ho, dtype=np.float32)

    in_maps = _shard_inputs(x, hidden, W_ih, b_ih, W_ho, b_ho)
    res = _run(in_maps)
    output = np.concatenate(
        [r["outT"].T.astype(np.float32) for r in res.results], axis=0) + b_ho
    new_hidden = np.concatenate(
        [r["nhT"].T.astype(np.float32) for r in res.results], axis=0)
    return output, new_hidden


# revision 24
# speedup vs baseline: 1.0085x; 1.0009x over previous
"""Fused RNN cell on 8 Trainium2 NeuronCores.

Reference computation (fp32):
    combined   = [x, hidden]                      [B=4096, I+H=4096]
    new_hidden = tanh(combined @ W_ih^T + b_ih)   [B, H=2048]
    output     = new_hidden @ W_ho^T + b_ho       [B, O=2048]
    returns (output, new_hidden)

Strategy: data-parallel over the batch — each of the 8 cores processes 512
batch rows with replicated weights; no collectives. All operand layout
transforms (transposes into PE-friendly [K-partition, free] form) happen on
the host so every device DMA is a fat, fully contiguous transfer:

    c   [128, 32, 512]      cL[ki, ko, b]       = combined[b, ko*128+ki]
    w1  [128, 32, 16, 128]  w1L[ki, ko, hc, h]  = W_ih[hc*128+h, ko*128+ki]
    w2  [128, 16, 16, 128]  w2L[hi, ho, oc, o]  = W_ho[oc*128+o, ho*128+hi]
    b1  [128, 16]           b1L[p, hc]          = b_ih[hc*128+p]

All matmul operands are fp16 (full PE rate, 1 col/cycle; fp32 PSUM
accumulation; rms rel err ~5e-4), which halves HBM traffic vs fp32 —
28 MB in + 4 MB out per core vs the ~166 us PE floor, so the kernel is
cleanly compute-bound.

DMA is split across the two HWDGE rings so neither saturates and the
first tiles land ASAP: the sync ring carries only weight loads (w1, w2);
the scalar (ACT) ring carries c loads, nh stores, and out stores. The
first four w1 slices are 1-ko (256 KB) so the PE can start ~1.5 us
earlier than with 2-ko slices; after the pipeline fills, 2-ko slices
(0.5 MB) keep trigger overhead low. Dummy matmuls at t=0 warm the PE
clock gate (HAM, ~3.4 us busy to reach 2.4 GHz) during the initial DMA
ramp, and a 1-element tanh preloads the ACT table set (~2.7 us) behind
the first c triggers.

mm1 drains: 8 back-to-back tanh ACTs per PSUM group (bias fused), then
the 8 nh store triggers — triggers never sit between ACT ops, so bank
turnaround stays at copy cadence and the next group never stalls. mm2
drains alternate DVE/ACT copies the same way; all out-store triggers
ride the scalar ring after the group's copies. mm2 groups are [8, 6, 2]
so only two small stores remain after the final matmul.
"""

import numpy as np

import concourse.bass as bass
import concourse.mybir as mybir
import concourse.tile as tile
from concourse import bacc, bass_utils

NCORES = 8
B, I, H, O = 4096, 2048, 2048, 2048
BC = B // NCORES          # 512 batch rows per core
K1 = I + H                # mm1 contraction dim, 4096
KO1 = K1 // 128           # 32 k-chunks for mm1
HC = H // 128             # 16 h-chunks
OC = O // 128             # 16 o-chunks
G = 8                     # h-chunks per PSUM group (8 banks)
P = 128
F32 = mybir.dt.float32
F16 = mybir.dt.float16
AF = mybir.ActivationFunctionType
NWARM = 32                # dummy matmuls covering the DMA ramp (~107ns each)


def _build():
    nc = bacc.Bacc("TRN2", target_bir_lowering=False)

    c = nc.dram_tensor("c", [P, KO1, BC], F16, kind="ExternalInput")
    w1 = nc.dram_tensor("w1", [P, KO1, HC, P], F16, kind="ExternalInput")
    b1 = nc.dram_tensor("b1", [P, HC], F32, kind="ExternalInput")
    w2 = nc.dram_tensor("w2", [P, HC, OC, P], F16, kind="ExternalInput")
    nhT = nc.dram_tensor("nhT", [H, BC], F16, kind="ExternalOutput")
    outT = nc.dram_tensor("outT", [O, BC], F16, kind="ExternalOutput")

    with tile.TileContext(nc) as tc:
        with tc.tile_pool(name="cpool", bufs=1) as cpool, \
             tc.tile_pool(name="wpool", bufs=10) as wpool, \
             tc.tile_pool(name="nhpool", bufs=1) as nhpool, \
             tc.tile_pool(name="opool", bufs=8) as opool, \
             tc.tile_pool(name="bpool", bufs=1) as bpool, \
             tc.tile_pool(name="ps", bufs=8, space="PSUM") as ps:

            # Warm-up operand first: the sooner the memset lands, the
            # sooner the PE warm-ups can start spinning up the clock.
            warm_sb = bpool.tile([P, P], mybir.dt.bfloat16)
            nc.vector.memset(warm_sb[:], 0.0)

            b1_sb = bpool.tile([P, HC], F32)

            c_sb = cpool.tile([P, KO1, BC], F16)
            nh_sb = nhpool.tile([P, HC, BC], F16)

            # Scalar-ring head: the first c chunks (needed by the first
            # real matmuls; the very first as its own small transfer so
            # matmuls can start before the rest lands) go out before the
            # ACT tanh-table preload blocks the queue for ~2.7 us.
            nc.scalar.dma_start(c_sb[:, 0:1], c[:, 0:1])
            nc.scalar.dma_start(c_sb[:, 1:2], c[:, 1:2])
            nc.scalar.dma_start(c_sb[:, 2:4], c[:, 2:4])
            nc.scalar.dma_start(c_sb[:, 4:6], c[:, 4:6])
            act_warm = bpool.tile([1, 1], F32)
            nc.scalar.activation(act_warm[:], warm_sb[:1, :1], AF.Tanh)
            for ko0 in range(6, KO1, 2):
                nc.scalar.dma_start(c_sb[:, ko0:ko0 + 2], c[:, ko0:ko0 + 2])
            # b_ih rides the scalar ring behind the c stream (it isn't
            # needed until the first group drains, ~60 us in); keeping it
            # off GpSimd SWDGE avoids the expensive dge_drain at teardown.
            nc.scalar.dma_start(b1_sb[:], b1[:])

            # mm1: nh^T[h, b] = tanh(W_ih @ combined^T + b_ih)
            # Two G-sized PSUM groups ping-pong across the 8 banks.
            for g in range(HC // G):
                psums = [ps.tile([P, BC], F32, tag="ps", name=f"ps{i}")
                         for i in range(G)]
                if g == 0:
                    # PE warm-up: HAM holds the PE at 1.2 GHz until ~3.4 us
                    # of busy time. Dummy matmuls (into the last bank this
                    # group will touch; start=True on the real group clears
                    # it) keep the PE active while the first tiles stream
                    # in, so real matmuls run near 2.4 GHz from the start.
                    for _ in range(NWARM):
                        nc.tensor.matmul(
                            psums[G - 1][:, :P], lhsT=warm_sb[:],
                            rhs=warm_sb[:],
                            start=True, stop=True, skip_group_check=True,
                        )
                    # 256 KB 1-ko slices throughout the first group: the
                    # ring pipeline is still filling, and halving the
                    # per-slice transfer keeps delivery ahead of the PE.
                    slices = [(ko, 1) for ko in range(KO1)]
                else:
                    slices = [(ko, 2) for ko in range(0, KO1, 2)]
                for ko0, kw in slices:
                    w1_sb = wpool.tile([P, 2, G, P], F16, tag="w")
                    nc.sync.dma_start(
                        w1_sb[:, :kw], w1[:, ko0:ko0 + kw, g * G:(g + 1) * G])
                    for kk in range(kw):
                        for i in range(G):
                            nc.tensor.matmul(
                                psums[i][:],
                                lhsT=w1_sb[:, kk, i],
                                rhs=c_sb[:, ko0 + kk],
                                start=(ko0 + kk == 0),
                                stop=(ko0 + kk == KO1 - 1),
                            )
                # Back-to-back tanhs first (bank turnaround at ACT copy
                # cadence), store triggers after.
                for i in range(G):
                    hc = g * G + i
                    nc.scalar.activation(
                        nh_sb[:, hc], psums[i][:], AF.Tanh,
                        bias=b1_sb[:, hc:hc + 1],
                    )
                for i in range(G):
                    hc = g * G + i
                    nc.scalar.dma_start(
                        nhT[hc * P:(hc + 1) * P, :], nh_sb[:, hc])

            # mm2: out^T[o, b] = W_ho @ nh^T (+ b_ho on host)
            # Groups of [8, 6, 2] o-chunks: consecutive groups ping-pong
            # through the 8 PSUM banks, and the final drain after the last
            # matmul is just two chunks on two parallel rings.
            for g0, gsz in ((0, 8), (8, 6), (14, 2)):
                psums = [ps.tile([P, BC], F32, tag="ps", name=f"ps{i}")
                         for i in range(gsz)]
                for ho0 in range(0, HC, 2):
                    w2_sb = wpool.tile([P, 2, G, P], F16, tag="w",
                                       name="w2_sb")[:, :, :gsz]
                    nc.sync.dma_start(
                        w2_sb[:], w2[:, ho0:ho0 + 2, g0:g0 + gsz])
                    for kk in range(2):
                        for i in range(gsz):
                            nc.tensor.matmul(
                                psums[i][:],
                                lhsT=w2_sb[:, kk, i],
                                rhs=nh_sb[:, ho0 + kk],
                                start=(ho0 + kk == 0),
                                stop=(ho0 + kk == HC - 1),
                            )
                # Alternate DVE/ACT copies back-to-back, then the store
                # triggers (scalar ring; the sync ring takes one of the two
                # tail stores so the final drain runs on parallel rings).
                o_sbs = []
                for i in range(gsz):
                    o_sb = opool.tile([P, BC], F16, tag="osb")
                    if i % 2:
                        nc.scalar.activation(o_sb[:], psums[i][:], AF.Copy)
                    else:
                        nc.vector.tensor_copy(o_sb[:], psums[i][:])
                    o_sbs.append(o_sb)
                for i in range(gsz):
                    oc = g0 + i
                    # Final two stores ride parallel rings: the last chunk
                    # takes the faster sync ring (w2 loads are done).
                    eng = nc.sync if (gsz == 2 and i == 1) else nc.scalar
                    eng.dma_start(outT[oc * P:(oc + 1) * P, :], o_sbs[i][:])

    nc.compile()
    return nc


def _shard_inputs(x, hidden, W_ih, b_ih, W_ho, b_ho):
    combined = np.concatenate([x, hidden], axis=1)  # [B, K1]
    w1L = np.ascontiguousarray(
        W_ih.reshape(HC, P, KO1, P).transpose(3, 2, 0, 1).astype(np.float16)
    )  # [ki, ko, hc, h]
    w2L = np.ascontiguousarray(
        W_ho.reshape(OC, P, HC, P).transpose(3, 2, 0, 1).astype(np.float16)
    )  # [hi, ho, oc, o]
    b1L = np.ascontiguousarray(b_ih.reshape(HC, P).T)
    in_maps = []
    for cix in range(NCORES):
        cc = combined[cix * BC:(cix + 1) * BC]  # [BC, K1]
        cL = np.ascontiguousarray(
            cc.reshape(BC, KO1, P).transpose(2, 1, 0).astype(np.float16))
        in_maps.append(
            {"c": cL, "w1": w1L, "b1": b1L, "w2": w2L}
        )
    return in_maps


def _run(in_maps, **kwargs):
    nc = _build()
    return bass_utils.run_bass_kernel_spmd(
        nc, in_maps, core_ids=list(range(NCORES)), **kwargs
    )


def kernel(x, hidden, W_ih, b_ih, W_ho, b_ho):
    x = np.asarray(x, dtype=np.float32)
    hidden = np.asarray(hidden, dtype=np.float32)
    W_ih = np.asarray(W_ih, dtype=np.float32)
    b_ih = np.asarray(b_ih, dtype=np.float32)
    W_ho = np.asarray(W_ho, dtype=np.float32)
    b_ho = np.asarray(b_ho, dtype=np.float32)

    in_maps = _shard_inputs(x, hidden, W_ih, b_ih, W_ho, b_ho)
    res = _run(in_maps)
    output = np.concatenate(
        [r["outT"].T.astype(np.float32) for r in res.results], axis=0) + b_ho
    new_hidden = np.concatenate(
        [r["nhT"].T.astype(np.float32) for r in res.results], axis=0)
    return output, new_hidden


# revision 25
# speedup vs baseline: 1.0091x; 1.0005x over previous
"""Fused RNN cell on 8 Trainium2 NeuronCores.

Reference computation (fp32):
    combined   = [x, hidden]                      [B=4096, I+H=4096]
    new_hidden = tanh(combined @ W_ih^T + b_ih)   [B, H=2048]
    output     = new_hidden @ W_ho^T + b_ho       [B, O=2048]
    returns (output, new_hidden)

Strategy: data-parallel over the batch — each of the 8 cores processes 512
batch rows with replicated weights; no collectives. All operand layout
transforms (transposes into PE-friendly [K-partition, free] form) happen on
the host so every device DMA is a fat, fully contiguous transfer:

    c   [128, 32, 512]      cL[ki, ko, b]       = combined[b, ko*128+ki]
    w1  [128, 32, 16, 128]  w1L[ki, ko, hc, h]  = W_ih[hc*128+h, ko*128+ki]
    w2  [128, 16, 16, 128]  w2L[hi, ho, oc, o]  = W_ho[oc*128+o, ho*128+hi]
    b1  [128, 16]           b1L[p, hc]          = b_ih[hc*128+p]

All matmul operands are fp16 (full PE rate, 1 col/cycle; fp32 PSUM
accumulation; rms rel err ~5e-4), which halves HBM traffic vs fp32 —
28 MB in + 4 MB out per core vs the ~166 us PE floor, so the kernel is
cleanly compute-bound.

DMA is split across the two HWDGE rings so neither saturates and the
first tiles land ASAP: the sync ring carries only weight loads (w1, w2);
the scalar (ACT) ring carries c loads, nh stores, and out stores. The
first four w1 slices are 1-ko (256 KB) so the PE can start ~1.5 us
earlier than with 2-ko slices; after the pipeline fills, 2-ko slices
(0.5 MB) keep trigger overhead low. Dummy matmuls at t=0 warm the PE
clock gate (HAM, ~3.4 us busy to reach 2.4 GHz) during the initial DMA
ramp, and a 1-element tanh preloads the ACT table set (~2.7 us) behind
the first c triggers.

mm1 drains: 8 back-to-back tanh ACTs per PSUM group (bias fused), then
the 8 nh store triggers — triggers never sit between ACT ops, so bank
turnaround stays at copy cadence and the next group never stalls. mm2
drains alternate DVE/ACT copies the same way; all out-store triggers
ride the scalar ring after the group's copies. mm2 groups are [8, 6, 2]
so only two small stores remain after the final matmul.
"""

import numpy as np

import concourse.bass as bass
import concourse.mybir as mybir
import concourse.tile as tile
from concourse import bacc, bass_utils

NCORES = 8
B, I, H, O = 4096, 2048, 2048, 2048
BC = B // NCORES          # 512 batch rows per core
K1 = I + H                # mm1 contraction dim, 4096
KO1 = K1 // 128           # 32 k-chunks for mm1
HC = H // 128             # 16 h-chunks
OC = O // 128             # 16 o-chunks
G = 8                     # h-chunks per PSUM group (8 banks)
P = 128
F32 = mybir.dt.float32
F16 = mybir.dt.float16
AF = mybir.ActivationFunctionType
NWARM = 34                # dummy matmuls covering the DMA ramp (~107ns each)


def _build():
    nc = bacc.Bacc("TRN2", target_bir_lowering=False)

    c = nc.dram_tensor("c", [P, KO1, BC], F16, kind="ExternalInput")
    w1 = nc.dram_tensor("w1", [P, KO1, HC, P], F16, kind="ExternalInput")
    b1 = nc.dram_tensor("b1", [P, HC], F32, kind="ExternalInput")
    w2 = nc.dram_tensor("w2", [P, HC, OC, P], F16, kind="ExternalInput")
    nhT = nc.dram_tensor("nhT", [H, BC], F16, kind="ExternalOutput")
    outT = nc.dram_tensor("outT", [O, BC], F16, kind="ExternalOutput")

    with tile.TileContext(nc) as tc:
        with tc.tile_pool(name="cpool", bufs=1) as cpool, \
             tc.tile_pool(name="wpool", bufs=10) as wpool, \
             tc.tile_pool(name="nhpool", bufs=1) as nhpool, \
             tc.tile_pool(name="opool", bufs=8) as opool, \
             tc.tile_pool(name="bpool", bufs=1) as bpool, \
             tc.tile_pool(name="ps", bufs=8, space="PSUM") as ps:

            # Warm-up operand first: the sooner the memset lands, the
            # sooner the PE warm-ups can start spinning up the clock.
            warm_sb = bpool.tile([P, P], mybir.dt.bfloat16)
            nc.vector.memset(warm_sb[:], 0.0)

            b1_sb = bpool.tile([P, HC], F32)

            c_sb = cpool.tile([P, KO1, BC], F16)
            nh_sb = nhpool.tile([P, HC, BC], F16)

            # Scalar-ring head: the first c chunks (needed by the first
            # real matmuls; the very first as its own small transfer so
            # matmuls can start before the rest lands) go out before the
            # ACT tanh-table preload blocks the queue for ~2.7 us.
            nc.scalar.dma_start(c_sb[:, 0:1], c[:, 0:1])
            nc.scalar.dma_start(c_sb[:, 1:2], c[:, 1:2])
            nc.scalar.dma_start(c_sb[:, 2:4], c[:, 2:4])
            nc.scalar.dma_start(c_sb[:, 4:6], c[:, 4:6])
            act_warm = bpool.tile([1, 1], F32)
            nc.scalar.activation(act_warm[:], warm_sb[:1, :1], AF.Tanh)
            for ko0 in range(6, KO1, 2):
                nc.scalar.dma_start(c_sb[:, ko0:ko0 + 2], c[:, ko0:ko0 + 2])
            # b_ih rides the scalar ring behind the c stream (it isn't
            # needed until the first group drains, ~60 us in); keeping it
            # off GpSimd SWDGE avoids the expensive dge_drain at teardown.
            nc.scalar.dma_start(b1_sb[:], b1[:])

            # mm1: nh^T[h, b] = tanh(W_ih @ combined^T + b_ih)
            # Two G-sized PSUM groups ping-pong across the 8 banks.
            for g in range(HC // G):
                psums = [ps.tile([P, BC], F32, tag="ps", name=f"ps{i}")
                         for i in range(G)]
                if g == 0:
                    # PE warm-up: HAM holds the PE at 1.2 GHz until ~3.4 us
                    # of busy time. Dummy matmuls (into the last bank this
                    # group will touch; start=True on the real group clears
                    # it) keep the PE active while the first tiles stream
                    # in, so real matmuls run near 2.4 GHz from the start.
                    for _ in range(NWARM):
                        nc.tensor.matmul(
                            psums[G - 1][:, :P], lhsT=warm_sb[:],
                            rhs=warm_sb[:],
                            start=True, stop=True, skip_group_check=True,
                        )
                    # 256 KB 1-ko slices throughout the first group: the
                    # ring pipeline is still filling, and halving the
                    # per-slice transfer keeps delivery ahead of the PE.
                    slices = [(ko, 1) for ko in range(KO1)]
                else:
                    slices = [(ko, 2) for ko in range(0, KO1, 2)]
                for ko0, kw in slices:
                    w1_sb = wpool.tile([P, 2, G, P], F16, tag="w")
                    nc.sync.dma_start(
                        w1_sb[:, :kw], w1[:, ko0:ko0 + kw, g * G:(g + 1) * G])
                    for kk in range(kw):
                        for i in range(G):
                            nc.tensor.matmul(
                                psums[i][:],
                                lhsT=w1_sb[:, kk, i],
                                rhs=c_sb[:, ko0 + kk],
                                start=(ko0 + kk == 0),
                                stop=(ko0 + kk == KO1 - 1),
                            )
                # Back-to-back tanhs first (bank turnaround at ACT copy
                # cadence), store triggers after.
                for i in range(G):
                    hc = g * G + i
                    nc.scalar.activation(
                        nh_sb[:, hc], psums[i][:], AF.Tanh,
                        bias=b1_sb[:, hc:hc + 1],
                    )
                for i in range(G):
                    hc = g * G + i
                    nc.scalar.dma_start(
                        nhT[hc * P:(hc + 1) * P, :], nh_sb[:, hc])

            # mm2: out^T[o, b] = W_ho @ nh^T (+ b_ho on host)
            # Groups of [8, 6, 2] o-chunks: consecutive groups ping-pong
            # through the 8 PSUM banks, and the final drain after the last
            # matmul is just two chunks on two parallel rings.
            for g0, gsz in ((0, 8), (8, 6), (14, 2)):
                psums = [ps.tile([P, BC], F32, tag="ps", name=f"ps{i}")
                         for i in range(gsz)]
                for ho0 in range(0, HC, 2):
                    w2_sb = wpool.tile([P, 2, G, P], F16, tag="w",
                                       name="w2_sb")[:, :, :gsz]
                    nc.sync.dma_start(
                        w2_sb[:], w2[:, ho0:ho0 + 2, g0:g0 + gsz])
                    for kk in range(2):
                        for i in range(gsz):
                            nc.tensor.matmul(
                                psums[i][:],
                                lhsT=w2_sb[:, kk, i],
                                rhs=nh_sb[:, ho0 + kk],
                                start=(ho0 + kk == 0),
                                stop=(ho0 + kk == HC - 1),
                            )
                # Alternate DVE/ACT copies back-to-back, then the store
                # triggers (scalar ring; the sync ring takes one of the two
                # tail stores so the final drain runs on parallel rings).
                o_sbs = []
                for i in range(gsz):
                    o_sb = opool.tile([P, BC], F16, tag="osb")
                    if i % 2:
                        nc.scalar.activation(o_sb[:], psums[i][:], AF.Copy)
                    else:
                        nc.vector.tensor_copy(o_sb[:], psums[i][:])
                    o_sbs.append(o_sb)
                for i in range(gsz):
                    oc = g0 + i
                    # Final two stores ride parallel rings: the last chunk
                    # takes the faster sync ring (w2 loads are done).
                    eng = nc.sync if (gsz == 2 and i == 1) else nc.scalar
                    eng.dma_start(outT[oc * P:(oc + 1) * P, :], o_sbs[i][:])

    nc.compile()
    return nc


def _shard_inputs(x, hidden, W_ih, b_ih, W_ho, b_ho):
    combined = np.concatenate([x, hidden], axis=1)  # [B, K1]
    w1L = np.ascontiguousarray(
        W_ih.reshape(HC, P, KO1, P).transpose(3, 2, 0, 1).astype(np.float16)
    )  # [ki, ko, hc, h]
    w2L = np.ascontiguousarray(
        W_ho.reshape(OC, P, HC, P).transpose(3, 2, 0, 1).astype(np.float16)
    )  # [hi, ho, oc, o]
    b1L = np.ascontiguousarray(b_ih.reshape(HC, P).T)
    in_maps = []
    for cix in range(NCORES):
        cc = combined[cix * BC:(cix + 1) * BC]  # [BC, K1]
        cL = np.ascontiguousarray(
            cc.reshape(BC, KO1, P).transpose(2, 1, 0).astype(np.float16))
        in_maps.append(
            {"c": cL, "w1": w1L, "b1": b1L, "w2": w2L}
        )
    return in_maps


def _run(in_maps, **kwargs):
    nc = _build()
    return bass_utils.run_bass_kernel_spmd(
        nc, in_maps, core_ids=list(range(NCORES)), **kwargs
    )


def kernel(x, hidden, W_ih, b_ih, W_ho, b_ho):
    x = np.asarray(x, dtype=np.float32)
    hidden = np.asarray(hidden, dtype=np.float32)
    W_ih = np.asarray(W_ih, dtype=np.float32)
    b_ih = np.asarray(b_ih, dtype=np.float32)
    W_ho = np.asarray(W_ho, dtype=np.float32)
    b_ho = np.asarray(b_ho, dtype=np.float32)

    in_maps = _shard_inputs(x, hidden, W_ih, b_ih, W_ho, b_ho)
    res = _run(in_maps)
    output = np.concatenate(
        [r["outT"].T.astype(np.float32) for r in res.results], axis=0) + b_ho
    new_hidden = np.concatenate(
        [r["nhT"].T.astype(np.float32) for r in res.results], axis=0)
    return output, new_hidden
